# revision 15
# baseline (speedup 1.0000x reference)
import numpy as np

import concourse.bass as bass
import concourse.bacc as bacc
import concourse.mybir as mybir
import concourse.tile as tile
from concourse.bass_utils import run_bass_kernel_spmd

B, T, C, H, D = 2, 2048, 1024, 16, 64
NCORES = 8
HPC = 4
CL = HPC * D
TS = 512
NTB = T // 128
NKC = C // 128
SM_SCALE = 1.0 / 8.0
GROUPS = [[0, 1, 2, 3], [4, 5, 6, 7]]

f32 = mybir.dt.float32
bf16 = mybir.dt.bfloat16
Exp = mybir.ActivationFunctionType.Exp

TRACE = False
TRACE_KWARGS = {}
LAST_RESULTS = None

_cached_nc = None


def _emit(nc, tc):
    xT_ext = nc.dram_tensor("xT", [C, T], bf16, kind="ExternalInput")
    wqk_ext = nc.dram_tensor("wqk", [C, 2 * CL], bf16, kind="ExternalInput")
    wv_ext = nc.dram_tensor("wv", [C, CL], bf16, kind="ExternalInput")
    wp_ext = nc.dram_tensor("wp", [CL, C], bf16, kind="ExternalInput")
    bqk_ext = nc.dram_tensor("bqk", [128, 4], f32, kind="ExternalInput")
    bv_ext = nc.dram_tensor("bv", [1, CL], bf16, kind="ExternalInput")
    out_ext = nc.dram_tensor("out", [TS, C], bf16, kind="ExternalOutput")
    pb = [nc.dram_tensor(f"pb{s}", [TS, C], bf16) for s in range(4)]
    rs = [[nc.dram_tensor(f"rs{s}_{c}", [64, C], bf16) for c in range(2)]
          for s in range(4)]
    warm_in = nc.dram_tensor("warm_in", [1, 128], bf16)
    warm_out = nc.dram_tensor("warm_out", [1, 128], bf16)

    with tc.tile_pool(name="persist", bufs=1) as pp, \
         tc.tile_pool(name="work", bufs=3) as wk, \
         tc.tile_pool(name="psum", bufs=2, space="PSUM") as psum:

        wt = pp.tile([1, 128], bf16, tag="wt")
        nc.gpsimd.memset(wt[:], 0.0)
        nc.sync.dma_start(out=warm_in[:], in_=wt[:])
        nc.gpsimd.collective_compute(
            "AllReduce", mybir.AluOpType.add, replica_groups=GROUPS,
            ins=[warm_in[:]], outs=[warm_out[:]])

        bqk = pp.tile([128, 4], f32, tag="bqk")
        nc.sync.dma_start(out=bqk[:], in_=bqk_ext[:])
        bv = pp.tile([1, CL], bf16, tag="bv")
        nc.sync.dma_start(out=bv[:], in_=bv_ext[:])
        ones = pp.tile([1, TS], bf16, tag="ones")
        nc.gpsimd.memset(ones[:], 1.0)

        xts, wqk, wv = [], [], []
        for kc in range(NKC):
            tw = pp.tile([128, 2 * CL], bf16, tag=f"wqk{kc}", name=f"wqk{kc}")
            nc.sync.dma_start(out=tw[:], in_=wqk_ext[128 * kc:128 * (kc + 1), :])
            wqk.append(tw)
            tx = pp.tile([128, T], bf16, tag=f"xt{kc}", name=f"xt{kc}")
            nc.sync.dma_start(out=tx[:], in_=xT_ext[128 * kc:128 * (kc + 1), :])
            xts.append(tx)
        for kc in range(NKC):
            t_ = pp.tile([128, CL], bf16, tag=f"wv{kc}", name=f"wv{kc}")
            nc.sync.dma_start(out=t_[:], in_=wv_ext[128 * kc:128 * (kc + 1), :])
            wv.append(t_)
        wp = []
        for kb in range(2):
            t_ = pp.tile([128, C], bf16, tag=f"wp{kb}", name=f"wp{kb}")
            nc.sync.dma_start(out=t_[:], in_=wp_ext[128 * kb:128 * (kb + 1), :])
            wp.append(t_)

        qk_sb = [pp.tile([128, T], bf16, tag=f"qk{mb}", name=f"qk{mb}")
                 for mb in range(4)]
        va = [pp.tile([128, HPC * (D + 1)], bf16, tag=f"va{tb}", name=f"va{tb}")
              for tb in range(NTB)]
        yn = [pp.tile([128, T], bf16, tag=f"yn{kb}", name=f"yn{kb}")
              for kb in range(2)]

        for mb in range(4):
            for s in range(4):
                pt = psum.tile([128, TS], f32, tag="mm", bufs=2)
                for kc in range(NKC):
                    nc.tensor.matmul(
                        pt[:],
                        lhsT=wqk[kc][:, 128 * mb:128 * (mb + 1)],
                        rhs=xts[kc][:, TS * s:TS * (s + 1)],
                        start=(kc == 0), stop=(kc == NKC - 1))
                nc.vector.tensor_scalar_add(
                    qk_sb[mb][:, TS * s:TS * (s + 1)], pt[:],
                    bqk[:, mb:mb + 1])

        for tb in range(NTB):
            for h in range(HPC):
                nc.gpsimd.memset(va[tb][:, 65 * h + 64:65 * h + 65], 1.0)
            pv = psum.tile([128, TS], f32, tag="mm", bufs=2)
            for kc in range(NKC):
                nc.tensor.matmul(
                    pv[:, 0:CL],
                    lhsT=xts[kc][:, 128 * tb:128 * (tb + 1)],
                    rhs=wv[kc][:],
                    start=(kc == 0), stop=False)
            nc.tensor.matmul(
                pv[:, 0:CL],
                lhsT=ones[0:1, 0:128],
                rhs=bv[0:1, :],
                start=False, stop=True)
            dst = va[tb][:].rearrange("p (h e) -> p h e", e=D + 1)[:, :, 0:D]
            src = pv[:, 0:CL].rearrange("p (h d) -> p h d", d=D)
            nc.vector.tensor_copy(dst, src)

        for s in (3, 2, 0, 1):
            nkb = 4 * s + 4
            for hp in (0, 2):
                hs = (hp, hp + 1)
                pos = [64 * (h % 2) for h in hs]
                qhs = [qk_sb[h // 2][64 * (h % 2):64 * (h % 2) + 64, :]
                       for h in hs]
                khs = [qk_sb[2 + h // 2][64 * (h % 2):64 * (h % 2) + 64, :]
                       for h in hs]
                yts = [psum.tile([D + 1, TS], f32, tag="yt", bufs=2,
                                 name=f"yt{h}")
                       for h in hs]
                for kb0 in range(0, nkb, 2):
                    sts, ests = [], []
                    for i in range(2):
                        st = psum.tile([128, 2 * TS], f32, tag="st",
                                       bufs=2, name=f"st{i}")
                        for j in range(2):
                            kb = kb0 + j
                            jd = kb - 4 * s
                            off = 128 * jd if jd >= 0 else 0
                            nc.tensor.matmul(
                                st[:, TS * j + off:TS * (j + 1)],
                                lhsT=khs[i][:, 128 * kb:128 * (kb + 1)],
                                rhs=qhs[i][:, TS * s + off:TS * (s + 1)],
                                start=True, stop=True)
                        sts.append(st)
                    for i in range(2):
                        est = wk.tile([128, 2 * TS], bf16, tag="est",
                                      bufs=4, name=f"est{i}")
                        nc.scalar.activation(est[:], sts[i][:], Exp,
                                             scale=SM_SCALE)
                        ests.append(est)
                    for i in range(2):
                        for j in range(2):
                            kb = kb0 + j
                            jd = kb - 4 * s
                            off = 128 * jd if jd >= 0 else 0
                            if jd >= 0:
                                nc.gpsimd.affine_select(
                                    out=ests[i][:, TS * j + off:TS * (j + 1)],
                                    in_=ests[i][:, TS * j + off:TS * (j + 1)],
                                    compare_op=mybir.AluOpType.is_ge,
                                    fill=0.0,
                                    base=TS * s + off - 128 * kb,
                                    channel_multiplier=-1,
                                    pattern=[[1, TS - off]])
                            nc.tensor.matmul(
                                yts[i][:, off:TS],
                                lhsT=va[kb][:, 65 * hs[i]:65 * hs[i] + 65],
                                rhs=ests[i][:, TS * j + off:TS * (j + 1)],
                                start=(kb == 0), stop=(kb == nkb - 1))
                for i in range(2):
                    rs_sb = wk.tile([1, TS], f32, tag="rs_sb")
                    nc.vector.tensor_copy(rs_sb[:], yts[i][D:D + 1, :])
                    rc = wk.tile([1, TS], f32, tag="rc")
                    nc.vector.reciprocal_approx_fast(rc[:], rs_sb[:])
                    bc = wk.tile([D, TS], f32, tag="bc")
                    nc.gpsimd.partition_broadcast(bc[:], rc[:])
                    nc.vector.tensor_mul(
                        yn[hs[i] // 2][pos[i]:pos[i] + 64,
                                       TS * s:TS * (s + 1)],
                        yts[i][0:D, :], bc[:])

            for c in range(2):
                for tbl in (2 * c, 2 * c + 1):
                    tb = 4 * s + tbl
                    for half in range(2):
                        pj = psum.tile([128, TS], f32, tag="mm", bufs=2)
                        for kb in range(2):
                            nc.tensor.matmul(
                                pj[:],
                                lhsT=yn[kb][:, 128 * tb:128 * (tb + 1)],
                                rhs=wp[kb][:, TS * half:TS * (half + 1)],
                                start=(kb == 0), stop=(kb == 1))
                        ps = wk.tile([128, TS], bf16, tag="po")
                        nc.vector.tensor_copy(ps[:], pj[:])
                        nc.sync.dma_start(
                            out=pb[s][128 * tbl:128 * (tbl + 1),
                                      TS * half:TS * (half + 1)],
                            in_=ps[:])
                nc.gpsimd.collective_compute(
                    "ReduceScatter",
                    mybir.AluOpType.add,
                    replica_groups=GROUPS,
                    ins=[pb[s][256 * c:256 * (c + 1), :]],
                    outs=[rs[s][c][:]],
                )

        for s in (3, 2, 1, 0):
            for c in range(2):
                nc.scalar.dma_start(
                    out=out_ext[128 * s + 64 * c:128 * s + 64 * (c + 1), :],
                    in_=rs[s][c][:])


def _build():
    global _cached_nc
    if _cached_nc is not None:
        return _cached_nc
    nc = bacc.Bacc("TRN2", target_bir_lowering=False, debug=False,
                   num_devices=NCORES)
    with tile.TileContext(nc) as tc:
        _emit(nc, tc)
    nc.compile()
    _cached_nc = nc
    return nc


def kernel(x, w_attn, b_attn, w_proj, b_proj):
    global LAST_RESULTS
    nc = _build()
    np_bf16 = mybir.dt.np(bf16)

    x = np.asarray(x, np.float32)
    w_attn = np.asarray(w_attn, np.float32)
    b_attn = np.asarray(b_attn, np.float32)
    w_proj = np.asarray(w_proj, np.float32)
    b_proj = np.asarray(b_proj, np.float32)

    xT = [np.ascontiguousarray(x[b].T).astype(np_bf16) for b in range(B)]

    in_maps = []
    for c in range(NCORES):
        b = c // 4
        g = c % 4
        cols = slice(CL * g, CL * (g + 1))
        wqk_c = np.concatenate(
            [w_attn[:, cols], w_attn[:, C + CL * g:C + CL * (g + 1)]],
            axis=1).astype(np_bf16)
        wv_c = np.ascontiguousarray(
            w_attn[:, 2 * C + CL * g:2 * C + CL * (g + 1)]).astype(np_bf16)
        wp_c = np.ascontiguousarray(w_proj[cols, :]).astype(np_bf16)
        bqk_c = np.ascontiguousarray(np.concatenate(
            [b_attn[cols], b_attn[C + CL * g:C + CL * (g + 1)]]
        ).reshape(4, 128).T).astype(np.float32)
        bv_c = b_attn[2 * C + CL * g:2 * C + CL * (g + 1)].reshape(
            1, -1).astype(np_bf16)
        in_maps.append({
            "xT": xT[b], "wqk": wqk_c, "wv": wv_c, "wp": wp_c,
            "bqk": bqk_c, "bv": bv_c,
        })

    res = run_bass_kernel_spmd(nc, in_maps, core_ids=list(range(NCORES)),
                               trace=TRACE, **TRACE_KWARGS)
    LAST_RESULTS = res

    y = np.empty((B, T, C), np.float32)
    for b in range(B):
        for r in range(4):
            shard = res.results[4 * b + r]["out"].astype(np.float32)
            for s in range(4):
                for c in range(2):
                    y[b][TS * s + 256 * c + 64 * r:
                         TS * s + 256 * c + 64 * (r + 1)] = \
                        shard[128 * s + 64 * c:128 * s + 64 * (c + 1)]
    y += b_proj[None, None, :]
    return y


# revision 32
# speedup vs baseline: 1.1286x; 1.1286x over previous
import numpy as np

import concourse.bass as bass
import concourse.bacc as bacc
import concourse.mybir as mybir
import concourse.tile as tile
from concourse.tile import add_dep_helper
from concourse.bass_utils import run_bass_kernel_spmd

B, T, C, H, D = 2, 2048, 1024, 16, 64
NCORES = 8
HPC = 4
CL = HPC * D
TS = 512
NTB = T // 128
NKC = C // 128
SM_SCALE = 1.0 / 8.0
GROUPS = [[0, 1, 2, 3], [4, 5, 6, 7]]

f32 = mybir.dt.float32
bf16 = mybir.dt.bfloat16
Exp = mybir.ActivationFunctionType.Exp

TRACE = False
TRACE_KWARGS = {}
LAST_RESULTS = None

_cached_nc = None


def _emit(nc, tc):
    xT_ext = nc.dram_tensor("xT", [C, T], bf16, kind="ExternalInput")
    wqk_ext = nc.dram_tensor("wqk", [C, 2 * CL], bf16, kind="ExternalInput")
    wv_ext = nc.dram_tensor("wv", [C, CL], bf16, kind="ExternalInput")
    wp_ext = nc.dram_tensor("wp", [CL, C], bf16, kind="ExternalInput")
    bqk_ext = nc.dram_tensor("bqk", [128, 4], f32, kind="ExternalInput")
    bv_ext = nc.dram_tensor("bv", [1, CL], bf16, kind="ExternalInput")
    out_ext = nc.dram_tensor("out", [TS, C], bf16, kind="ExternalOutput")
    pb = [nc.dram_tensor(f"pb{s}", [TS, C], bf16) for s in range(4)]
    rs = [[nc.dram_tensor(f"rs{s}_0", [128, C], bf16)] for s in range(4)]
    rs[1] = [nc.dram_tensor("rs1_0a", [64, C], bf16),
             nc.dram_tensor("rs1_1a", [64, C], bf16)]
    warm_in = nc.dram_tensor("warm_in", [1, 128], bf16)
    warm_out = nc.dram_tensor("warm_out", [1, 128], bf16)

    with tc.tile_pool(name="persist", bufs=1) as pp, \
         tc.tile_pool(name="work", bufs=3) as wk, \
         tc.tile_pool(name="psum", bufs=2, space="PSUM") as psum:

        wt = pp.tile([1, 128], bf16, tag="wt")
        nc.gpsimd.memset(wt[:], 0.0)
        nc.sync.dma_start(out=warm_in[:], in_=wt[:])
        nc.gpsimd.collective_compute(
            "AllReduce", mybir.AluOpType.add, replica_groups=GROUPS,
            ins=[warm_in[:]], outs=[warm_out[:]])

        bqk = pp.tile([128, 4], f32, tag="bqk")
        nc.sync.dma_start(out=bqk[:], in_=bqk_ext[:])
        bv = pp.tile([1, CL], bf16, tag="bv")
        nc.sync.dma_start(out=bv[:], in_=bv_ext[:])
        bvb = pp.tile([128, CL], bf16, tag="bvb")
        nc.gpsimd.partition_broadcast(bvb[:], bv[:])
        ones = pp.tile([1, TS], bf16, tag="ones")
        nc.gpsimd.memset(ones[:], 1.0)

        xts, wqk, wv = [], [], []
        for kc in range(NKC):
            tw = pp.tile([128, 2 * CL], bf16, tag=f"wqk{kc}", name=f"wqk{kc}")
            nc.sync.dma_start(out=tw[:], in_=wqk_ext[128 * kc:128 * (kc + 1), :])
            wqk.append(tw)
            tx = pp.tile([128, T], bf16, tag=f"xt{kc}", name=f"xt{kc}")
            nc.sync.dma_start(out=tx[:], in_=xT_ext[128 * kc:128 * (kc + 1), :])
            xts.append(tx)
        for kc in range(NKC):
            t_ = pp.tile([128, CL], bf16, tag=f"wv{kc}", name=f"wv{kc}")
            nc.sync.dma_start(out=t_[:], in_=wv_ext[128 * kc:128 * (kc + 1), :])
            wv.append(t_)
        wp = []
        for kb in range(2):
            t_ = pp.tile([128, C], bf16, tag=f"wp{kb}", name=f"wp{kb}")
            nc.sync.dma_start(out=t_[:], in_=wp_ext[128 * kb:128 * (kb + 1), :])
            wp.append(t_)

        qk_sb = [pp.tile([128, T], bf16, tag=f"qk{mb}", name=f"qk{mb}")
                 for mb in range(4)]
        va = [pp.tile([128, HPC * (D + 1)], bf16, tag=f"va{tb}", name=f"va{tb}")
              for tb in range(NTB)]
        yn = [pp.tile([128, T], bf16, tag=f"yn{kb}", name=f"yn{kb}")
              for kb in range(2)]
        qd = [None] * HPC
        kd = [None] * HPC

        def dup_qk(hp):
            for h in (hp, hp + 1):
                po = 64 * (h % 2)
                qsrc = qk_sb[h // 2][po:po + 64, :]
                ksrc = qk_sb[2 + h // 2][po:po + 64, :]
                qt = pp.tile([128, T], bf16, tag=f"qd{h}", name=f"qd{h}")
                kt = pp.tile([128, T], bf16, tag=f"kd{h}", name=f"kd{h}")
                nc.vector.tensor_copy(qt[0:64, :], qsrc)
                nc.vector.tensor_copy(qt[64:128, :], qsrc)
                nc.vector.tensor_copy(kt[0:64, :], ksrc)
                nc.vector.tensor_copy(kt[64:128, :], ksrc)
                qd[h] = qt
                kd[h] = kt

        def phase1_mb(mb):
            for s in range(4):
                pt = psum.tile([128, TS], f32, tag="u", bufs=3,
                               padded_shape=[128, 2 * TS], name="pt")
                for kc in range(NKC):
                    nc.tensor.matmul(
                        pt[:],
                        lhsT=wqk[kc][:, 128 * mb:128 * (mb + 1)],
                        rhs=xts[kc][:, TS * s:TS * (s + 1)],
                        start=(kc == 0), stop=(kc == NKC - 1))
                nc.vector.tensor_scalar_add(
                    qk_sb[mb][:, TS * s:TS * (s + 1)], pt[:],
                    bqk[:, mb:mb + 1])

        def phase2_va():
            for tb in range(NTB):
                for h in range(HPC):
                    nc.gpsimd.memset(va[tb][:, 65 * h + 64:65 * h + 65], 1.0)
                pv = psum.tile([128, TS], f32, tag="u", bufs=3,
                               padded_shape=[128, 2 * TS], name="pv")
                for kc in range(NKC):
                    nc.tensor.matmul(
                        pv[:, 0:CL],
                        lhsT=xts[kc][:, 128 * tb:128 * (tb + 1)],
                        rhs=wv[kc][:],
                        start=(kc == 0), stop=(kc == NKC - 1))
                dst = va[tb][:].rearrange(
                    "p (h e) -> p h e", e=D + 1)[:, :, 0:D]
                src = pv[:, 0:CL].rearrange("p (h d) -> p h d", d=D)
                nc.vector.tensor_add(
                    dst, src,
                    bvb[:].rearrange("p (h d) -> p h d", d=D))

        def attn(s, hp):
            nkb = 4 * s + 4
            hs = (hp, hp + 1)
            pos = [64 * (h % 2) for h in hs]
            yts = [psum.tile([D + 1, TS], f32, tag="yt", bufs=2,
                             name=f"yt{h}") for h in hs]
            for kb0 in range(0, nkb, 2):
                sts, ests = [], []
                for i in range(2):
                    st = psum.tile([128, 2 * TS], f32, tag="u",
                                   bufs=3, name=f"st{i}")
                    for j in range(2):
                        kb = kb0 + j
                        jd = kb - 4 * s
                        off = 128 * jd if jd >= 0 else 0
                        rg = 64 * j
                        nc.tensor.matmul(
                            st[:, TS * j + off:TS * (j + 1)],
                            lhsT=kd[hs[i]][rg:rg + 64,
                                           128 * kb:128 * (kb + 1)],
                            rhs=qd[hs[i]][rg:rg + 64,
                                          TS * s + off:TS * (s + 1)],
                            start=True, stop=True,
                            tile_position=(rg, 0))
                    sts.append(st)
                for i in range(2):
                    est = wk.tile([128, 2 * TS], bf16, tag="est",
                                  bufs=6, name=f"est{i}")
                    nc.scalar.activation(est[:], sts[i][:], Exp,
                                         scale=SM_SCALE)
                    ests.append(est)
                for i in range(2):
                    for j in range(2):
                        kb = kb0 + j
                        jd = kb - 4 * s
                        off = 128 * jd if jd >= 0 else 0
                        if jd >= 0:
                            nc.gpsimd.affine_select(
                                out=ests[i][:, TS * j + off:TS * (j + 1)],
                                in_=ests[i][:, TS * j + off:TS * (j + 1)],
                                compare_op=mybir.AluOpType.is_ge,
                                fill=0.0,
                                base=TS * s + off - 128 * kb,
                                channel_multiplier=-1,
                                pattern=[[1, TS - off]])
                        nc.tensor.matmul(
                            yts[i][:, off:TS],
                            lhsT=va[kb][:, 65 * hs[i]:65 * hs[i] + 65],
                            rhs=ests[i][:, TS * j + off:TS * (j + 1)],
                            start=(kb == 0), stop=(kb == nkb - 1))
            for i in range(2):
                rs_sb = wk.tile([1, TS], f32, tag="rs_sb", name="rs_sb")
                nc.vector.tensor_copy(rs_sb[:], yts[i][D:D + 1, :])
                rc = wk.tile([1, TS], f32, tag="rc", name="rc")
                nc.vector.reciprocal_approx_fast(rc[:], rs_sb[:])
                bc = wk.tile([D, TS], f32, tag="bc", name="bc")
                nc.gpsimd.partition_broadcast(bc[:], rc[:])
                nc.vector.tensor_mul(
                    yn[hs[i] // 2][pos[i]:pos[i] + 64, TS * s:TS * (s + 1)],
                    yts[i][0:D, :], bc[:])

        state = {"last_pb_dma": None}

        def proj_rs(s, nch):
            ntb_per = 4 // nch
            for ci in range(nch):
                for tbl in range(ci * ntb_per, (ci + 1) * ntb_per):
                    tb = 4 * s + tbl
                    for half in range(2):
                        pj = psum.tile([128, TS], f32, tag="u", bufs=3,
                                       padded_shape=[128, 2 * TS], name="pj")
                        for kb in range(2):
                            nc.tensor.matmul(
                                pj[:],
                                lhsT=yn[kb][:, 128 * tb:128 * (tb + 1)],
                                rhs=wp[kb][:, TS * half:TS * (half + 1)],
                                start=(kb == 0), stop=(kb == 1))
                        ps = wk.tile([128, TS], bf16, tag="po", name="ps")
                        nc.vector.tensor_copy(ps[:], pj[:])
                        state["last_pb_dma"] = nc.sync.dma_start(
                            out=pb[s][128 * tbl:128 * (tbl + 1),
                                      TS * half:TS * (half + 1)],
                            in_=ps[:])
                n = TS // nch
                nc.gpsimd.collective_compute(
                    "ReduceScatter",
                    mybir.AluOpType.add,
                    replica_groups=GROUPS,
                    ins=[pb[s][n * ci:n * (ci + 1), :]],
                    outs=[rs[s][ci][:]],
                )

        phase1_mb(0)
        phase1_mb(2)
        phase2_va()
        dup_qk(0)
        attn(0, 0)
        phase1_mb(1)
        phase1_mb(3)
        dup_qk(2)
        attn(0, 2)
        proj_rs(0, 1)
        for s_ in (3, 2):
            attn(s_, 0)
            attn(s_, 2)
            proj_rs(s_, 1)
        attn(1, 0)
        attn(1, 2)
        proj_rs(1, 2)

        for s in (3, 2, 0, 1):
            nsh = len(rs[s])
            n = 128 // nsh
            for c in range(nsh):
                od = nc.sync.dma_start(
                    out=out_ext[128 * s + n * c:128 * s + n * (c + 1), :],
                    in_=rs[s][c][:])
                add_dep_helper(od.ins, state["last_pb_dma"].ins, sync=False,
                               reason="keep output DMAs at stream tail")


def _build():
    global _cached_nc
    if _cached_nc is not None:
        return _cached_nc
    nc = bacc.Bacc("TRN2", target_bir_lowering=False, debug=False,
                   num_devices=NCORES)
    with tile.TileContext(nc) as tc:
        _emit(nc, tc)
    nc.compile()
    _cached_nc = nc
    return nc


def kernel(x, w_attn, b_attn, w_proj, b_proj):
    global LAST_RESULTS
    nc = _build()
    np_bf16 = mybir.dt.np(bf16)

    x = np.asarray(x, np.float32)
    w_attn = np.asarray(w_attn, np.float32)
    b_attn = np.asarray(b_attn, np.float32)
    w_proj = np.asarray(w_proj, np.float32)
    b_proj = np.asarray(b_proj, np.float32)

    xT = [np.ascontiguousarray(x[b].T).astype(np_bf16) for b in range(B)]

    in_maps = []
    for c in range(NCORES):
        b = c // 4
        g = c % 4
        cols = slice(CL * g, CL * (g + 1))
        wqk_c = np.concatenate(
            [w_attn[:, cols], w_attn[:, C + CL * g:C + CL * (g + 1)]],
            axis=1).astype(np_bf16)
        wv_c = np.ascontiguousarray(
            w_attn[:, 2 * C + CL * g:2 * C + CL * (g + 1)]).astype(np_bf16)
        wp_c = np.ascontiguousarray(w_proj[cols, :]).astype(np_bf16)
        bqk_c = np.ascontiguousarray(np.concatenate(
            [b_attn[cols], b_attn[C + CL * g:C + CL * (g + 1)]]
        ).reshape(4, 128).T).astype(np.float32)
        bv_c = b_attn[2 * C + CL * g:2 * C + CL * (g + 1)].reshape(
            1, -1).astype(np_bf16)
        in_maps.append({
            "xT": xT[b], "wqk": wqk_c, "wv": wv_c, "wp": wp_c,
            "bqk": bqk_c, "bv": bv_c,
        })

    res = run_bass_kernel_spmd(nc, in_maps, core_ids=list(range(NCORES)),
                               trace=TRACE, **TRACE_KWARGS)
    LAST_RESULTS = res

    y = np.empty((B, T, C), np.float32)
    for b in range(B):
        for r in range(4):
            shard = res.results[4 * b + r]["out"].astype(np.float32)
            for s in (0, 2, 3):
                y[b][TS * s + 128 * r:TS * s + 128 * (r + 1)] = \
                    shard[128 * s:128 * (s + 1)]
            for c in range(2):
                y[b][TS + 256 * c + 64 * r:TS + 256 * c + 64 * (r + 1)] = \
                    shard[128 + 64 * c:128 + 64 * (c + 1)]
    y += b_proj[None, None, :]
    return y


# revision 33
# speedup vs baseline: 1.1344x; 1.0051x over previous
import numpy as np

import concourse.bass as bass
import concourse.bacc as bacc
import concourse.mybir as mybir
import concourse.tile as tile
from concourse.tile import add_dep_helper
from concourse.bass_utils import run_bass_kernel_spmd

B, T, C, H, D = 2, 2048, 1024, 16, 64
NCORES = 8
HPC = 4
CL = HPC * D
TS = 512
NTB = T // 128
NKC = C // 128
SM_SCALE = 1.0 / 8.0
GROUPS = [[0, 1, 2, 3], [4, 5, 6, 7]]

f32 = mybir.dt.float32
bf16 = mybir.dt.bfloat16
Exp = mybir.ActivationFunctionType.Exp

TRACE = False
TRACE_KWARGS = {}
LAST_RESULTS = None

_cached_nc = None


def _emit(nc, tc):
    xT_ext = nc.dram_tensor("xT", [C, T], bf16, kind="ExternalInput")
    wqk_ext = nc.dram_tensor("wqk", [C, 2 * CL], bf16, kind="ExternalInput")
    wv_ext = nc.dram_tensor("wv", [C, CL], bf16, kind="ExternalInput")
    wp_ext = nc.dram_tensor("wp", [CL, C], bf16, kind="ExternalInput")
    bqk_ext = nc.dram_tensor("bqk", [128, 4], f32, kind="ExternalInput")
    bv_ext = nc.dram_tensor("bv", [1, CL], bf16, kind="ExternalInput")
    out_ext = nc.dram_tensor("out", [TS, C], bf16, kind="ExternalOutput")
    pb = [nc.dram_tensor(f"pb{s}", [TS, C], bf16) for s in range(4)]
    rs = [[nc.dram_tensor(f"rs{s}_0", [128, C], bf16)] for s in range(4)]
    rs[1] = [nc.dram_tensor("rs1_0a", [64, C], bf16),
             nc.dram_tensor("rs1_1a", [64, C], bf16)]
    warm_in = nc.dram_tensor("warm_in", [1, 128], bf16)
    warm_out = nc.dram_tensor("warm_out", [1, 128], bf16)

    with tc.tile_pool(name="persist", bufs=1) as pp, \
         tc.tile_pool(name="work", bufs=3) as wk, \
         tc.tile_pool(name="psum", bufs=2, space="PSUM") as psum:

        wt = pp.tile([1, 128], bf16, tag="wt")
        nc.gpsimd.memset(wt[:], 0.0)
        nc.sync.dma_start(out=warm_in[:], in_=wt[:])
        nc.gpsimd.collective_compute(
            "AllReduce", mybir.AluOpType.add, replica_groups=GROUPS,
            ins=[warm_in[:]], outs=[warm_out[:]])

        bqk = pp.tile([128, 4], f32, tag="bqk")
        nc.sync.dma_start(out=bqk[:], in_=bqk_ext[:])
        bv = pp.tile([1, CL], bf16, tag="bv")
        nc.sync.dma_start(out=bv[:], in_=bv_ext[:])
        bvb = pp.tile([128, CL], bf16, tag="bvb")
        nc.gpsimd.partition_broadcast(bvb[:], bv[:])

        xts, wqk, wv = [], [], []
        for kc in range(NKC):
            tw = pp.tile([128, 2 * CL], bf16, tag=f"wqk{kc}", name=f"wqk{kc}")
            nc.sync.dma_start(out=tw[:], in_=wqk_ext[128 * kc:128 * (kc + 1), :])
            wqk.append(tw)
            tx = pp.tile([128, T], bf16, tag=f"xt{kc}", name=f"xt{kc}")
            nc.sync.dma_start(out=tx[:, 0:TS],
                              in_=xT_ext[128 * kc:128 * (kc + 1), 0:TS])
            xts.append(tx)
        for kc in range(NKC):
            t_ = pp.tile([128, CL], bf16, tag=f"wv{kc}", name=f"wv{kc}")
            nc.sync.dma_start(out=t_[:], in_=wv_ext[128 * kc:128 * (kc + 1), :])
            wv.append(t_)
        for strip in range(1, 4):
            for kc in range(NKC):
                nc.sync.dma_start(
                    out=xts[kc][:, TS * strip:TS * (strip + 1)],
                    in_=xT_ext[128 * kc:128 * (kc + 1),
                               TS * strip:TS * (strip + 1)])
        wp = []
        for kb in range(2):
            t_ = pp.tile([128, C], bf16, tag=f"wp{kb}", name=f"wp{kb}")
            nc.sync.dma_start(out=t_[:], in_=wp_ext[128 * kb:128 * (kb + 1), :])
            wp.append(t_)

        qk_sb = [pp.tile([128, T], bf16, tag=f"qk{mb}", name=f"qk{mb}")
                 for mb in range(4)]
        va = [pp.tile([128, HPC * (D + 1)], bf16, tag=f"va{tb}", name=f"va{tb}")
              for tb in range(NTB)]
        yn = [pp.tile([128, T], bf16, tag=f"yn{kb}", name=f"yn{kb}")
              for kb in range(2)]
        qd = [None] * HPC
        kd = [None] * HPC

        def dup_qk(hp):
            for h in (hp, hp + 1):
                po = 64 * (h % 2)
                qsrc = qk_sb[h // 2][po:po + 64, :]
                ksrc = qk_sb[2 + h // 2][po:po + 64, :]
                qt = pp.tile([128, T], bf16, tag=f"qd{h}", name=f"qd{h}")
                kt = pp.tile([128, T], bf16, tag=f"kd{h}", name=f"kd{h}")
                nc.vector.tensor_copy(qt[0:64, :], qsrc)
                nc.vector.tensor_copy(qt[64:128, :], qsrc)
                nc.vector.tensor_copy(kt[0:64, :], ksrc)
                nc.vector.tensor_copy(kt[64:128, :], ksrc)
                qd[h] = qt
                kd[h] = kt

        def phase1_mb(mb):
            for s in range(4):
                pt = psum.tile([128, TS], f32, tag="u", bufs=3,
                               padded_shape=[128, 2 * TS], name="pt")
                for kc in range(NKC):
                    nc.tensor.matmul(
                        pt[:],
                        lhsT=wqk[kc][:, 128 * mb:128 * (mb + 1)],
                        rhs=xts[kc][:, TS * s:TS * (s + 1)],
                        start=(kc == 0), stop=(kc == NKC - 1))
                nc.vector.tensor_scalar_add(
                    qk_sb[mb][:, TS * s:TS * (s + 1)], pt[:],
                    bqk[:, mb:mb + 1])

        def phase2_va():
            for tb in range(NTB):
                for h in range(HPC):
                    nc.gpsimd.memset(va[tb][:, 65 * h + 64:65 * h + 65], 1.0)
                pv = psum.tile([128, TS], f32, tag="u", bufs=3,
                               padded_shape=[128, 2 * TS], name="pv")
                for kc in range(NKC):
                    nc.tensor.matmul(
                        pv[:, 0:CL],
                        lhsT=xts[kc][:, 128 * tb:128 * (tb + 1)],
                        rhs=wv[kc][:],
                        start=(kc == 0), stop=(kc == NKC - 1))
                dst = va[tb][:].rearrange(
                    "p (h e) -> p h e", e=D + 1)[:, :, 0:D]
                src = pv[:, 0:CL].rearrange("p (h d) -> p h d", d=D)
                nc.vector.tensor_add(
                    dst, src,
                    bvb[:].rearrange("p (h d) -> p h d", d=D))

        def attn(s, hp):
            nkb = 4 * s + 4
            hs = (hp, hp + 1)
            pos = [64 * (h % 2) for h in hs]
            yts = [psum.tile([D + 1, TS], f32, tag="yt", bufs=2,
                             name=f"yt{h}") for h in hs]
            for kb0 in range(0, nkb, 2):
                sts, ests = [], []
                for i in range(2):
                    st = psum.tile([128, 2 * TS], f32, tag="u",
                                   bufs=3, name=f"st{i}")
                    for j in range(2):
                        kb = kb0 + j
                        jd = kb - 4 * s
                        off = 128 * jd if jd >= 0 else 0
                        rg = 64 * j
                        nc.tensor.matmul(
                            st[:, TS * j + off:TS * (j + 1)],
                            lhsT=kd[hs[i]][rg:rg + 64,
                                           128 * kb:128 * (kb + 1)],
                            rhs=qd[hs[i]][rg:rg + 64,
                                          TS * s + off:TS * (s + 1)],
                            start=True, stop=True,
                            tile_position=(rg, 0))
                    sts.append(st)
                for i in range(2):
                    est = wk.tile([128, 2 * TS], bf16, tag="est",
                                  bufs=6, name=f"est{i}")
                    nc.scalar.activation(est[:], sts[i][:], Exp,
                                         scale=SM_SCALE)
                    ests.append(est)
                for i in range(2):
                    for j in range(2):
                        kb = kb0 + j
                        jd = kb - 4 * s
                        off = 128 * jd if jd >= 0 else 0
                        if jd >= 0:
                            nc.gpsimd.affine_select(
                                out=ests[i][:, TS * j + off:TS * (j + 1)],
                                in_=ests[i][:, TS * j + off:TS * (j + 1)],
                                compare_op=mybir.AluOpType.is_ge,
                                fill=0.0,
                                base=TS * s + off - 128 * kb,
                                channel_multiplier=-1,
                                pattern=[[1, TS - off]])
                        nc.tensor.matmul(
                            yts[i][:, off:TS],
                            lhsT=va[kb][:, 65 * hs[i]:65 * hs[i] + 65],
                            rhs=ests[i][:, TS * j + off:TS * (j + 1)],
                            start=(kb == 0), stop=(kb == nkb - 1))
            for i in range(2):
                rs_sb = wk.tile([1, TS], f32, tag="rs_sb", name="rs_sb")
                nc.vector.tensor_copy(rs_sb[:], yts[i][D:D + 1, :])
                rc = wk.tile([1, TS], f32, tag="rc", name="rc")
                nc.vector.reciprocal_approx_fast(rc[:], rs_sb[:])
                bc = wk.tile([D, TS], f32, tag="bc", name="bc")
                nc.gpsimd.partition_broadcast(bc[:], rc[:])
                nc.vector.tensor_mul(
                    yn[hs[i] // 2][pos[i]:pos[i] + 64, TS * s:TS * (s + 1)],
                    yts[i][0:D, :], bc[:])

        state = {"last_pb_dma": None}

        def proj_rs(s, nch):
            ntb_per = 4 // nch
            for ci in range(nch):
                for tbl in range(ci * ntb_per, (ci + 1) * ntb_per):
                    tb = 4 * s + tbl
                    for half in range(2):
                        pj = psum.tile([128, TS], f32, tag="u", bufs=3,
                                       padded_shape=[128, 2 * TS], name="pj")
                        for kb in range(2):
                            nc.tensor.matmul(
                                pj[:],
                                lhsT=yn[kb][:, 128 * tb:128 * (tb + 1)],
                                rhs=wp[kb][:, TS * half:TS * (half + 1)],
                                start=(kb == 0), stop=(kb == 1))
                        ps = wk.tile([128, TS], bf16, tag="po", name="ps")
                        nc.vector.tensor_copy(ps[:], pj[:])
                        state["last_pb_dma"] = nc.sync.dma_start(
                            out=pb[s][128 * tbl:128 * (tbl + 1),
                                      TS * half:TS * (half + 1)],
                            in_=ps[:])
                n = TS // nch
                nc.gpsimd.collective_compute(
                    "ReduceScatter",
                    mybir.AluOpType.add,
                    replica_groups=GROUPS,
                    ins=[pb[s][n * ci:n * (ci + 1), :]],
                    outs=[rs[s][ci][:]],
                )

        phase1_mb(0)
        phase1_mb(2)
        phase2_va()
        dup_qk(0)
        attn(0, 0)
        phase1_mb(1)
        phase1_mb(3)
        dup_qk(2)
        attn(0, 2)
        proj_rs(0, 1)
        for s_ in (3, 2):
            attn(s_, 0)
            attn(s_, 2)
            proj_rs(s_, 1)
        attn(1, 0)
        attn(1, 2)
        proj_rs(1, 2)

        for s in (3, 2, 0, 1):
            nsh = len(rs[s])
            n = 128 // nsh
            for c in range(nsh):
                od = nc.sync.dma_start(
                    out=out_ext[128 * s + n * c:128 * s + n * (c + 1), :],
                    in_=rs[s][c][:])
                add_dep_helper(od.ins, state["last_pb_dma"].ins, sync=False,
                               reason="keep output DMAs at stream tail")


def _build():
    global _cached_nc
    if _cached_nc is not None:
        return _cached_nc
    nc = bacc.Bacc("TRN2", target_bir_lowering=False, debug=False,
                   num_devices=NCORES)
    with tile.TileContext(nc) as tc:
        _emit(nc, tc)
    nc.compile()
    _cached_nc = nc
    return nc


def kernel(x, w_attn, b_attn, w_proj, b_proj):
    global LAST_RESULTS
    nc = _build()
    np_bf16 = mybir.dt.np(bf16)

    x = np.asarray(x, np.float32)
    w_attn = np.asarray(w_attn, np.float32)
    b_attn = np.asarray(b_attn, np.float32)
    w_proj = np.asarray(w_proj, np.float32)
    b_proj = np.asarray(b_proj, np.float32)

    xT = [np.ascontiguousarray(x[b].T).astype(np_bf16) for b in range(B)]

    in_maps = []
    for c in range(NCORES):
        b = c // 4
        g = c % 4
        cols = slice(CL * g, CL * (g + 1))
        wqk_c = np.concatenate(
            [w_attn[:, cols], w_attn[:, C + CL * g:C + CL * (g + 1)]],
            axis=1).astype(np_bf16)
        wv_c = np.ascontiguousarray(
            w_attn[:, 2 * C + CL * g:2 * C + CL * (g + 1)]).astype(np_bf16)
        wp_c = np.ascontiguousarray(w_proj[cols, :]).astype(np_bf16)
        bqk_c = np.ascontiguousarray(np.concatenate(
            [b_attn[cols], b_attn[C + CL * g:C + CL * (g + 1)]]
        ).reshape(4, 128).T).astype(np.float32)
        bv_c = b_attn[2 * C + CL * g:2 * C + CL * (g + 1)].reshape(
            1, -1).astype(np_bf16)
        in_maps.append({
            "xT": xT[b], "wqk": wqk_c, "wv": wv_c, "wp": wp_c,
            "bqk": bqk_c, "bv": bv_c,
        })

    res = run_bass_kernel_spmd(nc, in_maps, core_ids=list(range(NCORES)),
                               trace=TRACE, **TRACE_KWARGS)
    LAST_RESULTS = res

    y = np.empty((B, T, C), np.float32)
    for b in range(B):
        for r in range(4):
            shard = res.results[4 * b + r]["out"].astype(np.float32)
            for s in (0, 2, 3):
                y[b][TS * s + 128 * r:TS * s + 128 * (r + 1)] = \
                    shard[128 * s:128 * (s + 1)]
            for c in range(2):
                y[b][TS + 256 * c + 64 * r:TS + 256 * c + 64 * (r + 1)] = \
                    shard[128 + 64 * c:128 + 64 * (c + 1)]
    y += b_proj[None, None, :]
    return y


# revision 37
# speedup vs baseline: 1.2323x; 1.0863x over previous
import numpy as np

import concourse.bass as bass
import concourse.bacc as bacc
import concourse.mybir as mybir
import concourse.tile as tile
from concourse.tile import add_dep_helper
from concourse.bass_utils import run_bass_kernel_spmd

B, T, C, H, D = 2, 2048, 1024, 16, 64
NCORES = 8
HPC = 4
CL = HPC * D
TS = 512
NTB = T // 128
NKC = C // 128
SM_SCALE = 1.0 / 8.0
GROUPS = [[0, 1, 2, 3], [4, 5, 6, 7]]

f32 = mybir.dt.float32
bf16 = mybir.dt.bfloat16
Exp = mybir.ActivationFunctionType.Exp

TRACE = False
TRACE_KWARGS = {}
LAST_RESULTS = None

_cached_nc = None


def _emit(nc, tc):
    xT_ext = nc.dram_tensor("xT", [C, T], bf16, kind="ExternalInput")
    wqk_ext = nc.dram_tensor("wqk", [C, 2 * CL], bf16, kind="ExternalInput")
    wv_ext = nc.dram_tensor("wv", [C, CL], bf16, kind="ExternalInput")
    wp_ext = nc.dram_tensor("wp", [CL, C], bf16, kind="ExternalInput")
    bqk_ext = nc.dram_tensor("bqk", [128, 4], f32, kind="ExternalInput")
    bv_ext = nc.dram_tensor("bv", [1, CL], bf16, kind="ExternalInput")
    out_ext = nc.dram_tensor("out", [TS, C], bf16, kind="ExternalOutput")
    pb = [nc.dram_tensor(f"pb{s}", [TS, C], bf16) for s in range(4)]
    rs = [[nc.dram_tensor(f"rs{s}_0", [128, C], bf16)] for s in range(4)]
    rs[1] = [nc.dram_tensor("rs1_0a", [64, C], bf16),
             nc.dram_tensor("rs1_1a", [64, C], bf16)]
    warm_in = nc.dram_tensor("warm_in", [1, 128], bf16)
    warm_out = nc.dram_tensor("warm_out", [1, 128], bf16)

    with tc.tile_pool(name="persist", bufs=1) as pp, \
         tc.tile_pool(name="work", bufs=3) as wk, \
         tc.tile_pool(name="psum", bufs=2, space="PSUM") as psum:

        wt = pp.tile([1, 128], bf16, tag="wt")
        nc.gpsimd.memset(wt[:], 0.0)
        nc.sync.dma_start(out=warm_in[:], in_=wt[:])
        nc.gpsimd.collective_compute(
            "AllReduce", mybir.AluOpType.add, replica_groups=GROUPS,
            ins=[warm_in[:]], outs=[warm_out[:]])

        bqk = pp.tile([128, 4], f32, tag="bqk")
        nc.sync.dma_start(out=bqk[:], in_=bqk_ext[:])
        bv = pp.tile([1, CL], bf16, tag="bv")
        nc.sync.dma_start(out=bv[:], in_=bv_ext[:])
        bvb = pp.tile([128, CL], bf16, tag="bvb")
        nc.gpsimd.partition_broadcast(bvb[:], bv[:])

        xts, wqk, wv = [], [], []
        for kc in range(NKC):
            tw = pp.tile([128, 2 * CL], bf16, tag=f"wqk{kc}", name=f"wqk{kc}")
            nc.sync.dma_start(out=tw[:], in_=wqk_ext[128 * kc:128 * (kc + 1), :])
            wqk.append(tw)
            tx = pp.tile([128, T], bf16, tag=f"xt{kc}", name=f"xt{kc}")
            nc.sync.dma_start(out=tx[:, 0:TS],
                              in_=xT_ext[128 * kc:128 * (kc + 1), 0:TS])
            xts.append(tx)
        for kc in range(NKC):
            t_ = pp.tile([128, CL], bf16, tag=f"wv{kc}", name=f"wv{kc}")
            nc.sync.dma_start(out=t_[:], in_=wv_ext[128 * kc:128 * (kc + 1), :])
            wv.append(t_)
        for strip in range(1, 4):
            for kc in range(NKC):
                nc.sync.dma_start(
                    out=xts[kc][:, TS * strip:TS * (strip + 1)],
                    in_=xT_ext[128 * kc:128 * (kc + 1),
                               TS * strip:TS * (strip + 1)])
        wp = []
        for kb in range(2):
            t_ = pp.tile([128, C], bf16, tag=f"wp{kb}", name=f"wp{kb}")
            nc.sync.dma_start(out=t_[:], in_=wp_ext[128 * kb:128 * (kb + 1), :])
            wp.append(t_)

        qk_sb = [pp.tile([128, T], bf16, tag=f"qk{mb}", name=f"qk{mb}")
                 for mb in range(4)]
        va = [pp.tile([128, HPC * (D + 1)], bf16, tag=f"va{tb}", name=f"va{tb}")
              for tb in range(NTB)]
        yn = [pp.tile([128, T], bf16, tag=f"yn{kb}", name=f"yn{kb}")
              for kb in range(2)]
        qd = [None] * HPC
        kd = [None] * HPC

        def dup_qk(hp):
            for h in (hp, hp + 1):
                po = 64 * (h % 2)
                qsrc = qk_sb[h // 2][po:po + 64, :]
                ksrc = qk_sb[2 + h // 2][po:po + 64, :]
                qt = pp.tile([128, T], bf16, tag=f"qd{h}", name=f"qd{h}")
                kt = pp.tile([128, T], bf16, tag=f"kd{h}", name=f"kd{h}")
                nc.vector.tensor_copy(qt[0:64, :], qsrc)
                nc.vector.tensor_copy(qt[64:128, :], qsrc)
                nc.vector.tensor_copy(kt[0:64, :], ksrc)
                nc.vector.tensor_copy(kt[64:128, :], ksrc)
                qd[h] = qt
                kd[h] = kt

        def phase1_mb(mb):
            for s in range(4):
                pt = psum.tile([128, TS], f32, tag="u", bufs=3,
                               padded_shape=[128, 2 * TS], name="pt")
                for kc in range(NKC):
                    nc.tensor.matmul(
                        pt[:],
                        lhsT=wqk[kc][:, 128 * mb:128 * (mb + 1)],
                        rhs=xts[kc][:, TS * s:TS * (s + 1)],
                        start=(kc == 0), stop=(kc == NKC - 1))
                nc.vector.tensor_scalar_add(
                    qk_sb[mb][:, TS * s:TS * (s + 1)], pt[:],
                    bqk[:, mb:mb + 1])

        def phase2_va():
            for tb in range(NTB):
                for h in range(HPC):
                    nc.gpsimd.memset(va[tb][:, 65 * h + 64:65 * h + 65], 1.0)
                pv = psum.tile([128, TS], f32, tag="u", bufs=3,
                               padded_shape=[128, 2 * TS], name="pv")
                for kc in range(NKC):
                    nc.tensor.matmul(
                        pv[:, 0:CL],
                        lhsT=xts[kc][:, 128 * tb:128 * (tb + 1)],
                        rhs=wv[kc][:],
                        start=(kc == 0), stop=(kc == NKC - 1))
                dst = va[tb][:].rearrange(
                    "p (h e) -> p h e", e=D + 1)[:, :, 0:D]
                src = pv[:, 0:CL].rearrange("p (h d) -> p h d", d=D)
                nc.vector.tensor_add(
                    dst, src,
                    bvb[:].rearrange("p (h d) -> p h d", d=D))

        def attn_multi(units):
            ustates = []
            for (s, hp) in units:
                hs = (hp, hp + 1)
                ustates.append({
                    "s": s, "hs": hs,
                    "pos": [64 * (h % 2) for h in hs],
                    "nkb": 4 * s + 4,
                    "yts": [psum.tile([D + 1, TS], f32, tag="yt", bufs=2,
                                      name=f"yt{h}") for h in hs],
                })
            maxg = max(u["nkb"] for u in ustates) // 2
            for g in range(maxg):
                for u in ustates:
                    kb0 = 2 * g
                    if kb0 >= u["nkb"]:
                        continue
                    s, hs, nkb, yts = u["s"], u["hs"], u["nkb"], u["yts"]
                    sts, ests = [], []
                    for i in range(2):
                        st = psum.tile([128, 2 * TS], f32, tag="u",
                                       bufs=3, name=f"st{i}")
                        for j in range(2):
                            kb = kb0 + j
                            jd = kb - 4 * s
                            off = 128 * jd if jd >= 0 else 0
                            rg = 64 * j
                            nc.tensor.matmul(
                                st[:, TS * j + off:TS * (j + 1)],
                                lhsT=kd[hs[i]][rg:rg + 64,
                                               128 * kb:128 * (kb + 1)],
                                rhs=qd[hs[i]][rg:rg + 64,
                                              TS * s + off:TS * (s + 1)],
                                start=True, stop=True,
                                tile_position=(rg, 0))
                        sts.append(st)
                    for i in range(2):
                        est = wk.tile([128, 2 * TS], bf16, tag="est",
                                      bufs=8, name=f"est{i}")
                        nc.scalar.activation(est[:], sts[i][:], Exp,
                                             scale=SM_SCALE)
                        ests.append(est)
                    for i in range(2):
                        for j in range(2):
                            kb = kb0 + j
                            jd = kb - 4 * s
                            off = 128 * jd if jd >= 0 else 0
                            if jd >= 0:
                                nc.gpsimd.affine_select(
                                    out=ests[i][:,
                                                TS * j + off:TS * (j + 1)],
                                    in_=ests[i][:,
                                                TS * j + off:TS * (j + 1)],
                                    compare_op=mybir.AluOpType.is_ge,
                                    fill=0.0,
                                    base=TS * s + off - 128 * kb,
                                    channel_multiplier=-1,
                                    pattern=[[1, TS - off]])
                            nc.tensor.matmul(
                                yts[i][:, off:TS],
                                lhsT=va[kb][:, 65 * hs[i]:65 * hs[i] + 65],
                                rhs=ests[i][:, TS * j + off:TS * (j + 1)],
                                start=(kb == 0), stop=(kb == nkb - 1))
            for u in ustates:
                s, hs, pos, yts = u["s"], u["hs"], u["pos"], u["yts"]
                for i in range(2):
                    rs_sb = wk.tile([1, TS], f32, tag="rs_sb", name="rs_sb")
                    nc.vector.tensor_copy(rs_sb[:], yts[i][D:D + 1, :])
                    rc = wk.tile([1, TS], f32, tag="rc", name="rc")
                    nc.vector.reciprocal_approx_fast(rc[:], rs_sb[:])
                    bc = wk.tile([D, TS], f32, tag="bc", name="bc")
                    nc.gpsimd.partition_broadcast(bc[:], rc[:])
                    nc.vector.tensor_mul(
                        yn[hs[i] // 2][pos[i]:pos[i] + 64,
                                       TS * s:TS * (s + 1)],
                        yts[i][0:D, :], bc[:])

        def attn(s, hp):
            attn_multi([(s, hp)])

        state = {"last_pb_dma": None}

        def proj_rs(s, nch):
            ntb_per = 4 // nch
            for ci in range(nch):
                for tbl in range(ci * ntb_per, (ci + 1) * ntb_per):
                    tb = 4 * s + tbl
                    for half in range(2):
                        pj = psum.tile([128, TS], f32, tag="u", bufs=3,
                                       padded_shape=[128, 2 * TS], name="pj")
                        for kb in range(2):
                            nc.tensor.matmul(
                                pj[:],
                                lhsT=yn[kb][:, 128 * tb:128 * (tb + 1)],
                                rhs=wp[kb][:, TS * half:TS * (half + 1)],
                                start=(kb == 0), stop=(kb == 1))
                        ps = wk.tile([128, TS], bf16, tag="po", name="ps")
                        nc.vector.tensor_copy(ps[:], pj[:])
                        state["last_pb_dma"] = nc.sync.dma_start(
                            out=pb[s][128 * tbl:128 * (tbl + 1),
                                      TS * half:TS * (half + 1)],
                            in_=ps[:])
                n = TS // nch
                nc.gpsimd.collective_compute(
                    "ReduceScatter",
                    mybir.AluOpType.add,
                    replica_groups=GROUPS,
                    ins=[pb[s][n * ci:n * (ci + 1), :]],
                    outs=[rs[s][ci][:]],
                )

        phase1_mb(0)
        phase1_mb(2)
        phase2_va()
        dup_qk(0)
        attn(0, 0)
        phase1_mb(1)
        phase1_mb(3)
        dup_qk(2)
        attn(0, 2)
        proj_rs(0, 1)
        for s_ in (3, 2):
            attn(s_, 0)
            attn(s_, 2)
            proj_rs(s_, 1)
        attn(1, 0)
        attn(1, 2)
        proj_rs(1, 2)

        for s in (3, 2, 0, 1):
            nsh = len(rs[s])
            n = 128 // nsh
            for c in range(nsh):
                od = nc.sync.dma_start(
                    out=out_ext[128 * s + n * c:128 * s + n * (c + 1), :],
                    in_=rs[s][c][:])
                add_dep_helper(od.ins, state["last_pb_dma"].ins, sync=False,
                               reason="keep output DMAs at stream tail")


def _build():
    global _cached_nc
    if _cached_nc is not None:
        return _cached_nc
    nc = bacc.Bacc("TRN2", target_bir_lowering=False, debug=False,
                   num_devices=NCORES)
    with tile.TileContext(nc) as tc:
        _emit(nc, tc)
    nc.compile()
    _cached_nc = nc
    return nc


def kernel(x, w_attn, b_attn, w_proj, b_proj):
    global LAST_RESULTS
    nc = _build()
    np_bf16 = mybir.dt.np(bf16)

    x = np.asarray(x, np.float32)
    w_attn = np.asarray(w_attn, np.float32)
    b_attn = np.asarray(b_attn, np.float32)
    w_proj = np.asarray(w_proj, np.float32)
    b_proj = np.asarray(b_proj, np.float32)

    xT = [np.ascontiguousarray(x[b].T).astype(np_bf16) for b in range(B)]

    in_maps = []
    for c in range(NCORES):
        b = c // 4
        g = c % 4
        cols = slice(CL * g, CL * (g + 1))
        wqk_c = np.concatenate(
            [w_attn[:, cols], w_attn[:, C + CL * g:C + CL * (g + 1)]],
            axis=1).astype(np_bf16)
        wv_c = np.ascontiguousarray(
            w_attn[:, 2 * C + CL * g:2 * C + CL * (g + 1)]).astype(np_bf16)
        wp_c = np.ascontiguousarray(w_proj[cols, :]).astype(np_bf16)
        bqk_c = np.ascontiguousarray(np.concatenate(
            [b_attn[cols], b_attn[C + CL * g:C + CL * (g + 1)]]
        ).reshape(4, 128).T).astype(np.float32)
        bv_c = b_attn[2 * C + CL * g:2 * C + CL * (g + 1)].reshape(
            1, -1).astype(np_bf16)
        in_maps.append({
            "xT": xT[b], "wqk": wqk_c, "wv": wv_c, "wp": wp_c,
            "bqk": bqk_c, "bv": bv_c,
        })

    res = run_bass_kernel_spmd(nc, in_maps, core_ids=list(range(NCORES)),
                               trace=TRACE, **TRACE_KWARGS)
    LAST_RESULTS = res

    y = np.empty((B, T, C), np.float32)
    for b in range(B):
        for r in range(4):
            shard = res.results[4 * b + r]["out"].astype(np.float32)
            for s in (0, 2, 3):
                y[b][TS * s + 128 * r:TS * s + 128 * (r + 1)] = \
                    shard[128 * s:128 * (s + 1)]
            for c in range(2):
                y[b][TS + 256 * c + 64 * r:TS + 256 * c + 64 * (r + 1)] = \
                    shard[128 + 64 * c:128 + 64 * (c + 1)]
    y += b_proj[None, None, :]
    return y
